# revision 49
# baseline (speedup 1.0000x reference)
"""Trainium2 Bass kernel for the DiCNN (WaveNet-like) module.

Sharding: pure data parallelism — 4 batch items per core on 8 cores.
On-chip layout: channels on partitions, time on the free dim; the four
batch items are stacked as 4x32-partition bands (block-diag weights),
2x64 bands for the 64-channel causal layer.

Structure: a software-pipelined tile-major loop over eight 512-wide
time tiles. The output stage of tile t-1 (16 flipped 33->448 matmuls +
PSUM->SBUF bf16 copies + one packed store DMA per band) is woven into
tile t's body at every cross-engine rendezvous (z0-add, gate
activations, z1-residual, s0-relu), so the PE never idles waiting on
the vector/scalar chain. Narrow (64-col) warm-up matmuls ramp the PE
pstate during the input DMAs while spending almost none of the HAM
duty-cycle credit (~42k full-clock columns before the clock gate drops
to 4/8 duty); startup-critical input DMAs issue from the gpsimd queue,
which comes up ~5us before sync issues its first descriptor.

The final conv is "flipped": stationary = s1 data chunk [33,128]
(incl. a constant-ones row for the bias fold), moving = w_sk2^T
(+bias row) [33,448]; PSUM holds [t,co] — the output layout — so
results stream straight out with no transpose.

Output path: y is stored bf16 (upcast to fp32 on host; rel-err budget
2e-2 dwarfs bf16 rounding). The four 128-time chunks of each
(tile, band) use mod-4 interleaved stationary slices (chunk j covers
t = t0+4c+j at PSUM partition c), so the packed [128, 4, 448] SBUF
staging tile has each partition holding 4 consecutive DRAM rows =
3584B contiguous — one big-packet DMA per (tile, band) instead of
four row-fragmented ones. PSUM->SBUF copies rotate across the
vector/scalar/gpsimd engines.

All matmul operands are bf16 (fp32 PSUM accumulation). x is cast bf16
host-side so input loads can use the HWDGE DMA-transpose path.
"""

import numpy as np
import ml_dtypes

import concourse.bacc as bacc
import concourse.tile as tile
from concourse import mybir
from concourse.bass_utils import run_bass_kernel_spmd

BF16 = mybir.dt.bfloat16
FP32 = mybir.dt.float32

B, T, C_IN, HID, C_OUT, K = 32, 4096, 64, 32, 448, 2
N_CORES = 8
BPC = B // N_CORES          # batches per core = 4
TT = 512                    # time-tile size
NT = T // TT                # 8 tiles
XROWS = 4112                # 4097 rounded up to a multiple of 16 (xbar rows)
DELTA = 1                   # output-stage pipeline delay in tiles
N_WARMUP = 16               # dependency-free warm-up matmuls (64-col)
XPRE = 1040                 # x-prefix DMA columns (covers tiles 0-1)

AF = mybir.ActivationFunctionType
ALU = mybir.AluOpType

_cached_nc = None


def _f(x):
    return np.asarray(x, dtype=np.float32)


def _bf(x):
    return np.asarray(x, dtype=np.float32).astype(ml_dtypes.bfloat16)


def _tile4(v):
    return np.tile(_f(v).reshape(-1), 4).reshape(128, 1)


def prepare_weights(w_causal, b_causal, wd0, bd0, ws0, bs0, wo0, bo0,
                    wd1, bd1, ws1, bs1, wo1, bo1, w_sk1, b_sk1, w_sk2, b_sk2):
    """Host-side weight layout transforms (identical for every core)."""
    del wo1, bo1  # dead code: z after the last block is never used

    def diag4(w32):
        s = np.zeros((128, 128), np.float32)
        for i in range(4):
            s[32 * i:32 * i + 32, 32 * i:32 * i + 32] = w32
        return s

    wc = np.zeros((128, 4, 128), np.float32)
    for p in range(2):
        for k in range(2):
            wcT = _f(w_causal)[:, :, k].T
            s = np.zeros((128, 128), np.float32)
            s[0:64, 64 * p:64 * p + 32] = wcT
            s[64:128, 64 * p + 32:64 * p + 64] = wcT
            wc[:, 2 * p + k, :] = s

    wd = np.zeros((128, 4, 128), np.float32)
    for blk, w in enumerate((wd0, wd1)):
        for k in range(2):
            wd[:, 2 * blk + k, :] = diag4(_f(w)[:, :, k].T)

    wsr = np.zeros((128, 2, 128), np.float32)
    wsr[:, 0, :] = diag4(_f(ws0)[:, :, 0].T)
    wsr[:, 1, :] = diag4(_f(wo0)[:, :, 0].T)
    ws1d = diag4(_f(ws1)[:, :, 0].T)

    # w_sk1 pair stationaries: wsk1[:, q, :] covers batch bands 2q, 2q+1;
    # output cols 32 and 65 stay 0 so relu(0 + 1.0 bias) makes ones rows.
    wsk1 = np.zeros((128, 2, 97), np.float32)
    w1T = _f(w_sk1)[:, :, 0].T
    for q in range(2):
        wsk1[64 * q:64 * q + 32, q, 0:32] = w1T
        wsk1[64 * q + 32:64 * q + 64, q, 64:96] = w1T

    w2 = np.zeros((33, 448), np.float32)
    w2[0:32, :] = _f(w_sk2)[:, :, 0].T
    w2[32, :] = _f(b_sk2)

    bvecs = np.zeros((128, 6), np.float32)
    bvecs[:, 0] = _tile4(b_causal)[:, 0]
    bvecs[:, 1] = _tile4(bd0)[:, 0]
    bvecs[:, 2] = _tile4(bd1)[:, 0]
    bvecs[:, 3] = _tile4(bo0)[:, 0]
    bvecs[:, 4] = _tile4(_f(bs0) + _f(bs1))[:, 0]
    bvecs[0:32, 5] = _f(b_sk1)
    bvecs[32, 5] = 1.0
    bvecs[64:96, 5] = _f(b_sk1)
    bvecs[96, 5] = 1.0

    return dict(
        wc=_bf(wc), wd=_bf(wd), wsr=_bf(wsr), ws1d=_bf(ws1d),
        wsk1=_bf(wsk1), w2=_bf(w2), bvecs=np.ascontiguousarray(bvecs),
    )


def prepare_x(x, core):
    """Per-core pre-transposed input staging array [2, 128, XROWS] bf16.

    Column 0 is the causal zero pad (t=-1); column 1+t holds x[b, t, :]
    for the two batches of pair p stacked on the partition axis. Host
    pre-transposition lets the device use plain large-packet DMAs
    instead of the serializing xbar DMA-transpose path.
    """
    xT = np.zeros((2, 128, XROWS), ml_dtypes.bfloat16)
    xb = _bf(x)
    for p in range(2):
        xT[p, 0:64, 1:1 + T] = xb[4 * core + 2 * p].T
        xT[p, 64:128, 1:1 + T] = xb[4 * core + 2 * p + 1].T
    return xT


def build_nc():
    nc = bacc.Bacc("TRN2", target_bir_lowering=False, debug=False,
                   num_devices=N_CORES)

    xT_d = nc.dram_tensor("xT", [2, 128, XROWS], BF16, kind="ExternalInput")
    wc_d = nc.dram_tensor("wc", [128, 4, 128], BF16, kind="ExternalInput")
    wd_d = nc.dram_tensor("wd", [128, 4, 128], BF16, kind="ExternalInput")
    wsr_d = nc.dram_tensor("wsr", [128, 2, 128], BF16, kind="ExternalInput")
    ws1_d = nc.dram_tensor("ws1d", [128, 128], BF16, kind="ExternalInput")
    wsk1_d = nc.dram_tensor("wsk1", [128, 2, 97], BF16, kind="ExternalInput")
    w2_d = nc.dram_tensor("w2", [33, 448], BF16, kind="ExternalInput")
    bv_d = nc.dram_tensor("bvecs", [128, 6], FP32, kind="ExternalInput")
    # y rows pack 4 consecutive time steps: row r = t//4, col j*448+c.
    y_d = nc.dram_tensor("y", [BPC, T // 4, 4 * C_OUT], BF16,
                         kind="ExternalOutput")

    with tile.TileContext(nc) as tc:
        with (
            tc.tile_pool(name="const", bufs=1) as const,
            tc.tile_pool(name="persist", bufs=1) as persist,
            tc.tile_pool(name="act", bufs=3) as actp,
            tc.tile_pool(name="gtile", bufs=2) as gtp,
            tc.tile_pool(name="outbuf", bufs=8) as outbuf,
            tc.tile_pool(name="pz", bufs=1, space="PSUM") as pzp,
            tc.tile_pool(name="pg", bufs=3, space="PSUM") as pgp,
            tc.tile_pool(name="pout", bufs=2, space="PSUM") as poutp,
        ):
            # ---- constants. The startup-critical loads (wc for warmups,
            # the x prefix + wd for tile 0) go out on the vector HWDGE
            # queue, which comes up ~5us before sync issues its first
            # descriptor; everything else stays on sync.
            wc_s = const.tile([128, 4, 128], BF16)
            nc.gpsimd.dma_start(wc_s[:], wc_d.ap())
            x_s = [persist.tile([128, XROWS], BF16, tag=f"x{p}", name=f"x_s{p}")
                   for p in range(2)]
            nc.scalar.dma_start(x_s[0][:, 0:XPRE], xT_d[0, :, 0:XPRE])
            nc.gpsimd.dma_start(x_s[1][:, 0:XPRE], xT_d[1, :, 0:XPRE])
            wd_s = const.tile([128, 4, 128], BF16)
            nc.scalar.dma_start(wd_s[:], wd_d.ap())
            wsr_s = const.tile([128, 2, 128], BF16)
            nc.sync.dma_start(wsr_s[:], wsr_d.ap())
            ws1_s = const.tile([128, 128], BF16)
            nc.sync.dma_start(ws1_s[:], ws1_d.ap())
            wsk1_s = const.tile([128, 2, 97], BF16)
            nc.sync.dma_start(wsk1_s[:], wsk1_d.ap())
            # two copies of w2: base partition 0 (even s1 bands) and 64
            # (odd bands) — matmul needs lhsT/rhs base partitions equal.
            w2_s = const.tile([97, 448], BF16)
            nc.sync.dma_start(w2_s[0:33, :], w2_d.ap())
            nc.sync.dma_start(w2_s[64:97, :], w2_d.ap())
            bv_s = const.tile([128, 6], FP32)
            nc.sync.dma_start(bv_s[:], bv_d.ap())

            bcausal = bv_s[:, 0:1]
            bd_v = (bv_s[:, 1:2], bv_s[:, 2:3])
            bo0_v = bv_s[:, 3:4]
            bskip_v = bv_s[:, 4:5]
            bsk1_v = bv_s[:, 5:6]

            # ---- persistent activations ----
            for p in range(2):
                nc.sync.dma_start(x_s[p][:, XPRE:XROWS], xT_d[p, :, XPRE:XROWS])
            z0_s = persist.tile([128, 4100], BF16, tag="z0")
            nc.vector.memset(z0_s[:, 0:1], 0.0)
            z1_s = persist.tile([128, 4100], BF16, tag="z1")
            nc.vector.memset(z1_s[:, 0:2], 0.0)
            # s1 band pairs: rows 0-32 = band 2p (32 HID + ones row),
            # rows 64-96 = band 2p+1.
            s1p = [persist.tile([97, T], BF16, tag=f"s1_{p}", name=f"s1_{p}")
                   for p in range(2)]

            # ---- PE warm-up burst (overlaps the input DMAs) ----
            # Narrow (64/128-col) matmuls: the pstate ramp cares about
            # continuous busy TIME, while the HAM duty credit is spent per
            # COLUMN — narrow warm-ups ramp the clock almost for free.
            wu_t = persist.tile([128, TT], BF16, tag="wu")
            nc.vector.memset(wu_t[:], 0.0)
            hb_cnt = [0]

            def heartbeat(n, cols=128):
                """Dependency-free PE filler matmuls: keep the HAM activity
                window busy across short dependency stalls so the 2.4 GHz
                clock state is never lost."""
                for _ in range(n):
                    pwu = poutp.tile([128, TT], FP32, tag="po",
                                     name=f"pwu_{hb_cnt[0]}")
                    hb_cnt[0] += 1
                    nc.tensor.matmul(pwu[:, 0:cols], wc_s[:, 0, :],
                                     wu_t[:, 0:cols], start=True, stop=True)

            heartbeat(N_WARMUP, cols=64)

            # Copy-engine per paired out-stage copy (one per 2 chunks),
            # tuned so copies only sit in each engine's slack segments:
            # scalar is idle before tanh0 and between sig0/tanh1; vector
            # CASTs may not delay z1-STT but are free after g1-mul.
            COPY_ENG = "SSSSVVSV"

            def out_work(it, pattern=COPY_ENG):
                """Generator yielding one (mm + copy [+ band DMA]) step of
                tile `it`'s output stage per next() call.

                Chunk j's stationary is the mod-4 slice t = t0+4c+j, so
                PSUM partition c lands at packed row t0//4 + c, block j;
                each SBUF partition then holds 4 consecutive DRAM rows.
                """
                t0 = TT * it
                for q in range(4):
                    o_t = outbuf.tile([128, 4, C_OUT], BF16, tag="o",
                                      name=f"o_{it}_{q}")
                    r0 = 64 * (q % 2)
                    for h in range(2):
                        pt = poutp.tile([128, 2, 512], FP32, tag="po",
                                        name=f"po_{it}_{q}_{h}")
                        for jj in range(2):
                            j = 2 * h + jj
                            nc.tensor.matmul(
                                pt[:, jj, 0:C_OUT],
                                s1p[q // 2][r0:r0 + 33,
                                            t0 + j:t0 + j + 509:4],
                                w2_s[r0:r0 + 33, :], start=True, stop=True)
                            if jj == 0:
                                yield
                        # one cast covers both chunks (strided 2-bank read)
                        if pattern[2 * q + h] == "S":
                            nc.scalar.copy(o_t[:, 2 * h:2 * h + 2, :],
                                           pt[:, :, 0:C_OUT])
                        else:
                            nc.vector.tensor_copy(o_t[:, 2 * h:2 * h + 2, :],
                                                  pt[:, :, 0:C_OUT])
                        if h == 1:
                            nc.sync.dma_start(
                                y_d[q, 128 * it:128 * it + 128, :], o_t[:])
                        yield

            def weave(oq, n):
                """Advance the previous tile's out-stage by n steps (PE
                filler over the body's cross-engine rendezvous points)."""
                if oq is None:
                    heartbeat(1)
                    return
                for _ in range(n):
                    if next(oq, StopIteration) is StopIteration:
                        heartbeat(1)
                        break

            def emit_tile(it):
                """Body of tile `it`, with tile `it-1`'s out-stage woven
                into every dependency stall point."""
                t0 = TT * it
                oq = out_work(it - 1) if it >= 1 else None
                # -- causal conv: 4 accumulating MMs -> z0
                pz = pzp.tile([128, TT], FP32, tag="pz", name=f"pz_{it}")
                first = True
                for p in range(2):
                    rhs = (x_s[p][:, t0:t0 + TT], x_s[p][:, t0 + 1:t0 + 1 + TT])
                    for k in range(2):
                        nc.tensor.matmul(pz[:], wc_s[:, 2 * p + k, :], rhs[k],
                                         start=first, stop=(p == 1 and k == 1))
                        first = False
                nc.vector.tensor_scalar_add(z0_s[:, 1 + t0:1 + t0 + TT], pz[:],
                                            bcausal)
                weave(oq, 4)

                # -- block 0: g0 = gate(conv(z0, wd0, dil=1))
                pg0 = pgp.tile([128, TT], FP32, tag="ps", name=f"pg0_{it}")
                nc.tensor.matmul(pg0[:], wd_s[:, 0, :], z0_s[:, t0:t0 + TT],
                                 start=True, stop=False)
                nc.tensor.matmul(pg0[:], wd_s[:, 1, :],
                                 z0_s[:, 1 + t0:1 + t0 + TT],
                                 start=False, stop=True)
                a0 = actp.tile([128, TT], BF16, tag="a", name=f"a0_{it}")
                nc.scalar.activation(a0[:], pg0[:], AF.Tanh, bias=bd_v[0])
                b0 = actp.tile([128, TT], BF16, tag="b", name=f"b0_{it}")
                nc.scalar.activation(b0[:], pg0[:], AF.Sigmoid, bias=bd_v[0])
                g0 = gtp.tile([128, TT], BF16, tag="g0", name=f"g0_{it}")
                nc.vector.tensor_mul(g0[:], a0[:], b0[:])
                weave(oq, 4)

                # -- skip0 / res0 / z1. psA accumulates ws0@g0 now and
                # ws1@g1 later (open PSUM group across other banks), so
                # skip needs no copy and the head add comes for free.
                psA = pgp.tile([128, TT], FP32, tag="ps", name=f"psA_{it}")
                nc.tensor.matmul(psA[:], wsr_s[:, 0, :], g0[:],
                                 start=True, stop=False, skip_group_check=True)
                psB = pgp.tile([128, TT], FP32, tag="ps", name=f"psB_{it}")
                nc.tensor.matmul(psB[:], wsr_s[:, 1, :], g0[:],
                                 start=True, stop=True)
                nc.vector.scalar_tensor_tensor(
                    z1_s[:, 2 + t0:2 + t0 + TT], psB[:], bo0_v,
                    z0_s[:, 1 + t0:1 + t0 + TT], ALU.add, ALU.add)
                weave(oq, 2)

                # -- block 1: g1 = gate(conv(z1, wd1, dil=2))
                pg1 = pgp.tile([128, TT], FP32, tag="ps", name=f"pg1_{it}")
                nc.tensor.matmul(pg1[:], wd_s[:, 2, :], z1_s[:, t0:t0 + TT],
                                 start=True, stop=False)
                nc.tensor.matmul(pg1[:], wd_s[:, 3, :],
                                 z1_s[:, 2 + t0:2 + t0 + TT],
                                 start=False, stop=True)
                a1 = actp.tile([128, TT], BF16, tag="a", name=f"a1_{it}")
                nc.scalar.activation(a1[:], pg1[:], AF.Tanh, bias=bd_v[1])
                b1 = actp.tile([128, TT], BF16, tag="b", name=f"b1_{it}")
                nc.scalar.activation(b1[:], pg1[:], AF.Sigmoid, bias=bd_v[1])
                g1 = gtp.tile([128, TT], BF16, tag="g1", name=f"g1_{it}")
                nc.vector.tensor_mul(g1[:], a1[:], b1[:])
                weave(oq, 4)

                # -- head: psA += ws1@g1; s0 = relu(psA + (bs0+bs1))
                nc.tensor.matmul(psA[:], ws1_s[:], g1[:],
                                 start=False, stop=True, skip_group_check=True)
                s0_t = gtp.tile([128, TT], BF16, tag="s0", name=f"s0_{it}")
                nc.scalar.activation(s0_t[:], psA[:], AF.Relu, bias=bskip_v)
                weave(oq, 2)

                # -- s1 pairs: relu(wsk1@s0 + bias), ones rows via zero cols
                for q in range(2):
                    ps5 = pgp.tile([97, TT], FP32, tag="ps",
                                   name=f"ps5_{it}_{q}")
                    nc.tensor.matmul(ps5[:], wsk1_s[:, q, :], s0_t[:],
                                     start=True, stop=True)
                    nc.vector.tensor_scalar(s1p[q][:, t0:t0 + TT],
                                            ps5[:], bsk1_v[0:97], 0.0,
                                            ALU.add, ALU.max)
                if oq is not None:
                    for _ in oq:
                        pass

            for it in range(NT):
                emit_tile(it)
            # Tail out-stage has no body to hide in: alternate copy
            # engines so scalar and vector drain in parallel.
            for _ in out_work(NT - 1, pattern="SVSV" * 2):
                pass

    nc.compile()
    return nc


def get_nc():
    global _cached_nc
    if _cached_nc is None:
        _cached_nc = build_nc()
    return _cached_nc


def kernel(**inputs):
    nc = get_nc()
    w = prepare_weights(
        inputs["w_causal"], inputs["b_causal"],
        inputs["wd0"], inputs["bd0"], inputs["ws0"], inputs["bs0"],
        inputs["wo0"], inputs["bo0"],
        inputs["wd1"], inputs["bd1"], inputs["ws1"], inputs["bs1"],
        inputs["wo1"], inputs["bo1"],
        inputs["w_sk1"], inputs["b_sk1"], inputs["w_sk2"], inputs["b_sk2"])
    x = np.asarray(inputs["x"])
    in_maps = [{"xT": prepare_x(x, c), **w} for c in range(N_CORES)]
    res = run_bass_kernel_spmd(nc, in_maps, list(range(N_CORES)))
    out = np.concatenate(
        [np.asarray(res.results[c]["y"]).reshape(BPC, T, C_OUT)
         for c in range(N_CORES)], axis=0)
    return out.astype(np.float32)



# revision 51
# speedup vs baseline: 1.0703x; 1.0703x over previous
"""Trainium2 Bass kernel for the DiCNN (WaveNet-like) module.

Sharding: pure data parallelism — 4 batch items per core on 8 cores.
On-chip layout: channels on partitions, time on the free dim; the four
batch items are stacked as 4x32-partition bands (block-diag weights),
2x64 bands for the 64-channel causal layer.

Structure: a software-pipelined tile-major loop over eight 512-wide
time tiles. The output stage of tile t-1 (16 flipped 33->448 matmuls +
PSUM->SBUF bf16 copies + one packed store DMA per band) is woven into
tile t's body at every cross-engine rendezvous (z0-add, gate
activations, z1-residual, s0-relu), so the PE never idles waiting on
the vector/scalar chain. Narrow (64-col) warm-up matmuls ramp the PE
pstate during the input DMAs while spending almost none of the HAM
duty-cycle credit (~42k full-clock columns before the clock gate drops
to 4/8 duty); startup-critical input DMAs issue from the gpsimd queue,
which comes up ~5us before sync issues its first descriptor.

The final conv is "flipped": stationary = s1 data chunk [33,128]
(incl. a constant-ones row for the bias fold), moving = w_sk2^T
(+bias row) [33,448]; PSUM holds [t,co] — the output layout — so
results stream straight out with no transpose.

Output path: y is stored bf16 (upcast to fp32 on host; rel-err budget
2e-2 dwarfs bf16 rounding). The four 128-time chunks of each
(tile, band) use mod-4 interleaved stationary slices (chunk j covers
t = t0+4c+j at PSUM partition c), so the packed [128, 4, 448] SBUF
staging tile has each partition holding 4 consecutive DRAM rows =
3584B contiguous — one big-packet DMA per (tile, band) instead of
four row-fragmented ones. PSUM->SBUF copies rotate across the
vector/scalar/gpsimd engines.

All matmul operands are bf16 (fp32 PSUM accumulation). x is cast bf16
host-side so input loads can use the HWDGE DMA-transpose path.
"""

import numpy as np
import ml_dtypes

import concourse.bacc as bacc
import concourse.tile as tile
from concourse import mybir
from concourse.bass_utils import run_bass_kernel_spmd

BF16 = mybir.dt.bfloat16
FP32 = mybir.dt.float32

B, T, C_IN, HID, C_OUT, K = 32, 4096, 64, 32, 448, 2
N_CORES = 8
BPC = B // N_CORES          # batches per core = 4
TT = 512                    # time-tile size
NT = T // TT                # 8 tiles
XROWS = 4112                # 4097 rounded up to a multiple of 16 (xbar rows)
DELTA = 1                   # output-stage pipeline delay in tiles
N_WARMUP = 16               # dependency-free warm-up matmuls (64-col)
XPRE = 1040                 # x-prefix DMA columns (covers tiles 0-1)

AF = mybir.ActivationFunctionType
ALU = mybir.AluOpType

_cached_nc = None


def _f(x):
    return np.asarray(x, dtype=np.float32)


def _bf(x):
    return np.asarray(x, dtype=np.float32).astype(ml_dtypes.bfloat16)


def _tile4(v):
    return np.tile(_f(v).reshape(-1), 4).reshape(128, 1)


def prepare_weights(w_causal, b_causal, wd0, bd0, ws0, bs0, wo0, bo0,
                    wd1, bd1, ws1, bs1, wo1, bo1, w_sk1, b_sk1, w_sk2, b_sk2):
    """Host-side weight layout transforms (identical for every core)."""
    del wo1, bo1  # dead code: z after the last block is never used

    def diag4(w32):
        s = np.zeros((128, 128), np.float32)
        for i in range(4):
            s[32 * i:32 * i + 32, 32 * i:32 * i + 32] = w32
        return s

    wc = np.zeros((128, 4, 128), np.float32)
    for p in range(2):
        for k in range(2):
            wcT = _f(w_causal)[:, :, k].T
            s = np.zeros((128, 128), np.float32)
            s[0:64, 64 * p:64 * p + 32] = wcT
            s[64:128, 64 * p + 32:64 * p + 64] = wcT
            wc[:, 2 * p + k, :] = s

    wd = np.zeros((128, 4, 128), np.float32)
    for blk, w in enumerate((wd0, wd1)):
        for k in range(2):
            wd[:, 2 * blk + k, :] = diag4(_f(w)[:, :, k].T)

    wsr = np.zeros((128, 2, 128), np.float32)
    wsr[:, 0, :] = diag4(_f(ws0)[:, :, 0].T)
    wsr[:, 1, :] = diag4(_f(wo0)[:, :, 0].T)
    ws1d = diag4(_f(ws1)[:, :, 0].T)

    # w_sk1 pair stationaries: wsk1[:, q, :] covers batch bands 2q, 2q+1;
    # output cols 32 and 65 stay 0 so relu(0 + 1.0 bias) makes ones rows.
    wsk1 = np.zeros((128, 2, 97), np.float32)
    w1T = _f(w_sk1)[:, :, 0].T
    for q in range(2):
        wsk1[64 * q:64 * q + 32, q, 0:32] = w1T
        wsk1[64 * q + 32:64 * q + 64, q, 64:96] = w1T

    w2 = np.zeros((33, 448), np.float32)
    w2[0:32, :] = _f(w_sk2)[:, :, 0].T
    w2[32, :] = _f(b_sk2)

    bvecs = np.zeros((128, 6), np.float32)
    bvecs[:, 0] = _tile4(b_causal)[:, 0]
    bvecs[:, 1] = _tile4(bd0)[:, 0]
    bvecs[:, 2] = _tile4(bd1)[:, 0]
    bvecs[:, 3] = _tile4(bo0)[:, 0]
    bvecs[:, 4] = _tile4(_f(bs0) + _f(bs1))[:, 0]
    bvecs[0:32, 5] = _f(b_sk1)
    bvecs[32, 5] = 1.0
    bvecs[64:96, 5] = _f(b_sk1)
    bvecs[96, 5] = 1.0

    return dict(
        wc=_bf(wc), wd=_bf(wd), wsr=_bf(wsr), ws1d=_bf(ws1d),
        wsk1=_bf(wsk1), w2=_bf(w2), bvecs=np.ascontiguousarray(bvecs),
    )


def prepare_x(x, core):
    """Per-core pre-transposed input staging array [2, 128, XROWS] bf16.

    Column 0 is the causal zero pad (t=-1); column 1+t holds x[b, t, :]
    for the two batches of pair p stacked on the partition axis. Host
    pre-transposition lets the device use plain large-packet DMAs
    instead of the serializing xbar DMA-transpose path.
    """
    xT = np.zeros((2, 128, XROWS), ml_dtypes.bfloat16)
    xb = _bf(x)
    for p in range(2):
        xT[p, 0:64, 1:1 + T] = xb[4 * core + 2 * p].T
        xT[p, 64:128, 1:1 + T] = xb[4 * core + 2 * p + 1].T
    return xT


def build_nc():
    nc = bacc.Bacc("TRN2", target_bir_lowering=False, debug=False,
                   num_devices=N_CORES)

    xT_d = nc.dram_tensor("xT", [2, 128, XROWS], BF16, kind="ExternalInput")
    wc_d = nc.dram_tensor("wc", [128, 4, 128], BF16, kind="ExternalInput")
    wd_d = nc.dram_tensor("wd", [128, 4, 128], BF16, kind="ExternalInput")
    wsr_d = nc.dram_tensor("wsr", [128, 2, 128], BF16, kind="ExternalInput")
    ws1_d = nc.dram_tensor("ws1d", [128, 128], BF16, kind="ExternalInput")
    wsk1_d = nc.dram_tensor("wsk1", [128, 2, 97], BF16, kind="ExternalInput")
    w2_d = nc.dram_tensor("w2", [33, 448], BF16, kind="ExternalInput")
    bv_d = nc.dram_tensor("bvecs", [128, 6], FP32, kind="ExternalInput")
    # y rows pack 4 consecutive time steps: row r = t//4, col j*448+c.
    y_d = nc.dram_tensor("y", [BPC, T // 4, 4 * C_OUT], BF16,
                         kind="ExternalOutput")

    with tile.TileContext(nc) as tc:
        with (
            tc.tile_pool(name="const", bufs=1) as const,
            tc.tile_pool(name="persist", bufs=1) as persist,
            tc.tile_pool(name="act", bufs=3) as actp,
            tc.tile_pool(name="gtile", bufs=2) as gtp,
            tc.tile_pool(name="outbuf", bufs=8) as outbuf,
            tc.tile_pool(name="pz", bufs=1, space="PSUM") as pzp,
            tc.tile_pool(name="pg", bufs=3, space="PSUM") as pgp,
            tc.tile_pool(name="pout", bufs=4, space="PSUM") as poutp,
        ):
            # ---- constants. The startup-critical loads (wc for warmups,
            # the x prefix + wd for tile 0) go out on the vector HWDGE
            # queue, which comes up ~5us before sync issues its first
            # descriptor; everything else stays on sync.
            wc_s = const.tile([128, 4, 128], BF16)
            nc.gpsimd.dma_start(wc_s[:], wc_d.ap())
            x_s = [persist.tile([128, XROWS], BF16, tag=f"x{p}", name=f"x_s{p}")
                   for p in range(2)]
            nc.scalar.dma_start(x_s[0][:, 0:XPRE], xT_d[0, :, 0:XPRE])
            nc.gpsimd.dma_start(x_s[1][:, 0:XPRE], xT_d[1, :, 0:XPRE])
            wd_s = const.tile([128, 4, 128], BF16)
            nc.scalar.dma_start(wd_s[:], wd_d.ap())
            wsr_s = const.tile([128, 2, 128], BF16)
            nc.sync.dma_start(wsr_s[:], wsr_d.ap())
            ws1_s = const.tile([128, 128], BF16)
            nc.sync.dma_start(ws1_s[:], ws1_d.ap())
            wsk1_s = const.tile([128, 2, 97], BF16)
            nc.sync.dma_start(wsk1_s[:], wsk1_d.ap())
            # two copies of w2: base partition 0 (even s1 bands) and 64
            # (odd bands) — matmul needs lhsT/rhs base partitions equal.
            w2_s = const.tile([97, 448], BF16)
            nc.sync.dma_start(w2_s[0:33, :], w2_d.ap())
            nc.sync.dma_start(w2_s[64:97, :], w2_d.ap())
            bv_s = const.tile([128, 6], FP32)
            nc.sync.dma_start(bv_s[:], bv_d.ap())

            bcausal = bv_s[:, 0:1]
            bd_v = (bv_s[:, 1:2], bv_s[:, 2:3])
            bo0_v = bv_s[:, 3:4]
            bskip_v = bv_s[:, 4:5]
            bsk1_v = bv_s[:, 5:6]

            # ---- persistent activations ----
            for p in range(2):
                nc.sync.dma_start(x_s[p][:, XPRE:XROWS], xT_d[p, :, XPRE:XROWS])
            z0_s = persist.tile([128, 4100], BF16, tag="z0")
            nc.vector.memset(z0_s[:, 0:1], 0.0)
            z1_s = persist.tile([128, 4100], BF16, tag="z1")
            nc.vector.memset(z1_s[:, 0:2], 0.0)
            # s1 band pairs: rows 0-32 = band 2p (32 HID + ones row),
            # rows 64-96 = band 2p+1.
            s1p = [persist.tile([97, T], BF16, tag=f"s1_{p}", name=f"s1_{p}")
                   for p in range(2)]

            # ---- PE warm-up burst (overlaps the input DMAs) ----
            # Narrow (64/128-col) matmuls: the pstate ramp cares about
            # continuous busy TIME, while the HAM duty credit is spent per
            # COLUMN — narrow warm-ups ramp the clock almost for free.
            wu_t = persist.tile([128, TT], BF16, tag="wu")
            nc.vector.memset(wu_t[:], 0.0)
            hb_cnt = [0]

            def heartbeat(n, cols=128):
                """Dependency-free PE filler matmuls: keep the HAM activity
                window busy across short dependency stalls so the 2.4 GHz
                clock state is never lost."""
                for _ in range(n):
                    pwu = poutp.tile([128, TT], FP32, tag="po",
                                     name=f"pwu_{hb_cnt[0]}")
                    hb_cnt[0] += 1
                    nc.tensor.matmul(pwu[:, 0:cols], wc_s[:, 0, :],
                                     wu_t[:, 0:cols], start=True, stop=True)

            heartbeat(N_WARMUP, cols=64)

            # Copy-engine per out-stage step, tuned so copies only sit in
            # each engine's slack segments: scalar is idle before tanh0
            # and between sig0/tanh1; vector CASTs may not delay z1-STT
            # but are free after g1-mul and before the s1p stores.
            COPY_ENG = "SSSVSSSSVVVVVVVV"

            def out_work(it, pattern=COPY_ENG):
                """Generator yielding one (mm + copy [+ band DMA]) step of
                tile `it`'s output stage per next() call.

                Chunk j's stationary is the mod-4 slice t = t0+4c+j, so
                PSUM partition c lands at packed row t0//4 + c, block j;
                each SBUF partition then holds 4 consecutive DRAM rows.
                """
                t0 = TT * it
                for q in range(4):
                    o_t = outbuf.tile([128, 4, C_OUT], BF16, tag="o",
                                      name=f"o_{it}_{q}")
                    r0 = 64 * (q % 2)
                    for j in range(4):
                        po = poutp.tile([128, C_OUT], FP32, tag="po",
                                        name=f"po_{it}_{q}_{j}")
                        nc.tensor.matmul(
                            po[:],
                            s1p[q // 2][r0:r0 + 33, t0 + j:t0 + j + 509:4],
                            w2_s[r0:r0 + 33, :], start=True, stop=True)
                        if pattern[4 * q + j] == "S":
                            nc.scalar.copy(o_t[:, j, :], po[:])
                        else:
                            nc.vector.tensor_copy(o_t[:, j, :], po[:])
                        if j == 3:
                            nc.sync.dma_start(
                                y_d[q, 128 * it:128 * it + 128, :], o_t[:])
                        yield

            def weave(oq, n):
                """Advance the previous tile's out-stage by n steps (PE
                filler over the body's cross-engine rendezvous points)."""
                if oq is None:
                    heartbeat(1)
                    return
                for _ in range(n):
                    if next(oq, StopIteration) is StopIteration:
                        heartbeat(1)
                        break

            def emit_tile(it):
                """Body of tile `it`, with tile `it-1`'s out-stage woven
                into every dependency stall point."""
                t0 = TT * it
                oq = out_work(it - 1) if it >= 1 else None
                # -- causal conv: 4 accumulating MMs -> z0
                pz = pzp.tile([128, TT], FP32, tag="pz", name=f"pz_{it}")
                first = True
                for p in range(2):
                    rhs = (x_s[p][:, t0:t0 + TT], x_s[p][:, t0 + 1:t0 + 1 + TT])
                    for k in range(2):
                        nc.tensor.matmul(pz[:], wc_s[:, 2 * p + k, :], rhs[k],
                                         start=first, stop=(p == 1 and k == 1))
                        first = False
                nc.vector.tensor_scalar_add(z0_s[:, 1 + t0:1 + t0 + TT], pz[:],
                                            bcausal)
                weave(oq, 4)

                # -- block 0: g0 = gate(conv(z0, wd0, dil=1))
                pg0 = pgp.tile([128, TT], FP32, tag="ps", name=f"pg0_{it}")
                nc.tensor.matmul(pg0[:], wd_s[:, 0, :], z0_s[:, t0:t0 + TT],
                                 start=True, stop=False)
                nc.tensor.matmul(pg0[:], wd_s[:, 1, :],
                                 z0_s[:, 1 + t0:1 + t0 + TT],
                                 start=False, stop=True)
                a0 = actp.tile([128, TT], BF16, tag="a", name=f"a0_{it}")
                nc.scalar.activation(a0[:], pg0[:], AF.Tanh, bias=bd_v[0])
                b0 = actp.tile([128, TT], BF16, tag="b", name=f"b0_{it}")
                nc.scalar.activation(b0[:], pg0[:], AF.Sigmoid, bias=bd_v[0])
                g0 = gtp.tile([128, TT], BF16, tag="g0", name=f"g0_{it}")
                nc.vector.tensor_mul(g0[:], a0[:], b0[:])
                weave(oq, 4)

                # -- skip0 / res0 / z1. psA accumulates ws0@g0 now and
                # ws1@g1 later (open PSUM group across other banks), so
                # skip needs no copy and the head add comes for free.
                psA = pgp.tile([128, TT], FP32, tag="ps", name=f"psA_{it}")
                nc.tensor.matmul(psA[:], wsr_s[:, 0, :], g0[:],
                                 start=True, stop=False, skip_group_check=True)
                psB = pgp.tile([128, TT], FP32, tag="ps", name=f"psB_{it}")
                nc.tensor.matmul(psB[:], wsr_s[:, 1, :], g0[:],
                                 start=True, stop=True)
                nc.vector.scalar_tensor_tensor(
                    z1_s[:, 2 + t0:2 + t0 + TT], psB[:], bo0_v,
                    z0_s[:, 1 + t0:1 + t0 + TT], ALU.add, ALU.add)
                weave(oq, 2)

                # -- block 1: g1 = gate(conv(z1, wd1, dil=2))
                pg1 = pgp.tile([128, TT], FP32, tag="ps", name=f"pg1_{it}")
                nc.tensor.matmul(pg1[:], wd_s[:, 2, :], z1_s[:, t0:t0 + TT],
                                 start=True, stop=False)
                nc.tensor.matmul(pg1[:], wd_s[:, 3, :],
                                 z1_s[:, 2 + t0:2 + t0 + TT],
                                 start=False, stop=True)
                a1 = actp.tile([128, TT], BF16, tag="a", name=f"a1_{it}")
                nc.scalar.activation(a1[:], pg1[:], AF.Tanh, bias=bd_v[1])
                b1 = actp.tile([128, TT], BF16, tag="b", name=f"b1_{it}")
                nc.scalar.activation(b1[:], pg1[:], AF.Sigmoid, bias=bd_v[1])
                g1 = gtp.tile([128, TT], BF16, tag="g1", name=f"g1_{it}")
                nc.vector.tensor_mul(g1[:], a1[:], b1[:])
                weave(oq, 4)

                # -- head: psA += ws1@g1; s0 = relu(psA + (bs0+bs1))
                nc.tensor.matmul(psA[:], ws1_s[:], g1[:],
                                 start=False, stop=True, skip_group_check=True)
                s0_t = gtp.tile([128, TT], BF16, tag="s0", name=f"s0_{it}")
                nc.scalar.activation(s0_t[:], psA[:], AF.Relu, bias=bskip_v)
                weave(oq, 2)

                # -- s1 pairs: relu(wsk1@s0 + bias), ones rows via zero cols
                for q in range(2):
                    ps5 = pgp.tile([97, TT], FP32, tag="ps",
                                   name=f"ps5_{it}_{q}")
                    nc.tensor.matmul(ps5[:], wsk1_s[:, q, :], s0_t[:],
                                     start=True, stop=True)
                    nc.vector.tensor_scalar(s1p[q][:, t0:t0 + TT],
                                            ps5[:], bsk1_v[0:97], 0.0,
                                            ALU.add, ALU.max)
                if oq is not None:
                    for _ in oq:
                        pass

            for it in range(NT):
                emit_tile(it)
            # Tail out-stage has no body to hide in: alternate copy
            # engines so scalar and vector drain in parallel.
            for _ in out_work(NT - 1, pattern="SVSV" * 4):
                pass

    nc.compile()
    return nc


def get_nc():
    global _cached_nc
    if _cached_nc is None:
        _cached_nc = build_nc()
    return _cached_nc


def kernel(**inputs):
    nc = get_nc()
    w = prepare_weights(
        inputs["w_causal"], inputs["b_causal"],
        inputs["wd0"], inputs["bd0"], inputs["ws0"], inputs["bs0"],
        inputs["wo0"], inputs["bo0"],
        inputs["wd1"], inputs["bd1"], inputs["ws1"], inputs["bs1"],
        inputs["wo1"], inputs["bo1"],
        inputs["w_sk1"], inputs["b_sk1"], inputs["w_sk2"], inputs["b_sk2"])
    x = np.asarray(inputs["x"])
    in_maps = [{"xT": prepare_x(x, c), **w} for c in range(N_CORES)]
    res = run_bass_kernel_spmd(nc, in_maps, list(range(N_CORES)))
    out = np.concatenate(
        [np.asarray(res.results[c]["y"]).reshape(BPC, T, C_OUT)
         for c in range(N_CORES)], axis=0)
    return out.astype(np.float32)

